# revision 17
# baseline (speedup 1.0000x reference)
"""Trainium2 Bass kernel for relative-position attention (dense_transformer).

Reference computation (per batch element b):
    q = x @ Wq; k, v = split(x @ Wkv); heads of 64
    dots = (q k^T) * 64^-0.5
    pos[n, r]  = (q[n] . pos_table[512 + clip(n - r, -512, 512)]) * 64^-0.5
    out = softmax(dots + pos) @ v; concat heads; @ Wo + bo

Sharding: pure data-parallel over the batch (B=8 -> 8 NeuronCores).

Design (v2, "transposed softmax"):
  * logits are computed TRANSPOSED: L^T[r, n] = k q^T + P^T, with the key
    index r on PSUM partitions.  exp(L^T) goes straight to SBUF as the
    attn@v operand -- the [1024,1024] E matrix never touches DRAM and no
    DMA transpose is needed (the old kernel spent ~34 MB of HBM on that).
  * relative-position term: s_ext = q @ TR (windowed, 1152 per 128-row
    tile) is staged to DRAM in FP8 (x16 scale) and skew-gathered back as
    P tiles [n, r] (1KB contiguous runs).  P^T is produced by fp8
    transpose-matmuls (lhsT = P tile, rhs = I/128) accumulating directly
    into the logits PSUM, so the skew costs no extra elementwise pass.
  * softmax denominator: v gets a ones-column (65-wide per head); the
    attn@v matmul then emits Z = sum_r E^T[r, n] as psum row 64.  The
    1/Z normalisation happens once on the [64, 1024] output drain
    (reciprocal + gpsimd partition_broadcast + one DVE multiply).
  * HBM traffic/core ~24 MB vs ~75 MB for the v1 kernel.
"""

import numpy as np
import ml_dtypes

import concourse.bass as bass
from concourse import bacc
import concourse.mybir as mybir
from concourse.tile import TileContext
from concourse.bass_utils import run_bass_kernel_spmd

B, N, DIM = 8, 1024, 512
HEADS, DH, INNER = 8, 64, 512
NT = N // 128            # 8 row tiles of 128
WIN = 1152               # s_ext window width per row tile
SW = NT * WIN            # 9216 staged cols per head
SW2 = 2 * SW             # 18432: A|B combined staging row
SCALE = DH ** -0.5
BF = mybir.dt.bfloat16
F32 = mybir.dt.float32
F8 = mybir.dt.float8e4
BF_NP = ml_dtypes.bfloat16
F8_NP = ml_dtypes.float8_e4m3

EXP = mybir.ActivationFunctionType.Exp

_CACHE = {}
LAST_RESULTS = None


def _install_ntff_hook():
    """The image's antenv package lacks axon_hooks; provide it so
    run_bass_kernel_spmd(trace=True) can capture NTFF profiles."""
    import sys
    import types
    if "antenv.axon_hooks" in sys.modules:
        return
    try:
        from trn_agent_boot.trn_boot import _ntff_profile_via_ctypes
        hook = _ntff_profile_via_ctypes("/opt/axon/libaxon_pjrt.so")
    except Exception:
        hook = None
    mod = types.ModuleType("antenv.axon_hooks")
    mod._hook = hook
    mod.set_axon_ntff_profile_hook = lambda h: setattr(mod, "_hook", h)
    mod.get_axon_ntff_profile_hook = lambda: mod._hook
    sys.modules["antenv.axon_hooks"] = mod


def build():
    nc = bacc.Bacc("TRN2")

    xT = nc.dram_tensor("xT", [DIM, N], BF, kind="ExternalInput")
    Wq = nc.dram_tensor("Wq", [DIM, INNER], BF, kind="ExternalInput")
    Wk = nc.dram_tensor("Wk", [DIM, INNER], BF, kind="ExternalInput")
    Wv = nc.dram_tensor("Wv", [DIM, INNER], BF, kind="ExternalInput")
    Wo = nc.dram_tensor("Wo", [INNER, DIM], BF, kind="ExternalInput")
    bo_b = nc.dram_tensor("bo_b", [128, DIM], F32, kind="ExternalInput")
    TR8 = nc.dram_tensor("TR8", [128, 2048], F8, kind="ExternalInput")
    Id8 = nc.dram_tensor("Id8", [128, 128], F8, kind="ExternalInput")
    out = nc.dram_tensor("out", [N, DIM], F32, kind="ExternalOutput")

    with TileContext(nc) as tc:
        with (
            tc.tile_pool(name="persist", bufs=1) as persist,
            tc.tile_pool(name="work", bufs=3) as work,
            tc.tile_pool(name="stage", bufs=2) as stage,
            tc.tile_pool(name="pbp", bufs=4) as pbp,
            tc.tile_pool(name="etp", bufs=6) as etp,
            tc.tile_pool(name="ps", bufs=2, space="PSUM") as ps,
            tc.tile_pool(name="pso", bufs=2, space="PSUM") as pso,
            tc.tile_pool(name="sdram", bufs=3, space="DRAM") as sdram,
        ):
            # ---- persistent SBUF tensors ----
            xT_sb = [persist.tile([128, N], BF, name=f"xT{i}") for i in range(4)]
            Wq_sb = [persist.tile([128, INNER], BF, name=f"Wq{i}") for i in range(4)]
            Wk_sb = [persist.tile([128, INNER], BF, name=f"Wk{i}") for i in range(4)]
            Wv_sb = [persist.tile([128, INNER], BF, name=f"Wv{i}") for i in range(4)]
            Wo_sb = [persist.tile([128, DIM], BF, name=f"Wo{i}") for i in range(4)]
            TR_sb = persist.tile([128, 2048], F8, name="TRt")
            bo_sb = persist.tile([128, DIM], F32, name="bot")
            id_sb = persist.tile([128, 128], F8, name="idt")
            qT_sb = [persist.tile([128, N], BF, name=f"qT{i}") for i in range(4)]
            q8_sb = [persist.tile([128, N], F8, name=f"q8{i}") for i in range(4)]
            kT_sb = [persist.tile([128, N], BF, name=f"kT{i}") for i in range(4)]
            v_sb = [persist.tile([128, 8 * 65], BF, name=f"v{i}") for i in range(8)]
            aoT_sb = [persist.tile([128, N], BF, name=f"aoT{i}") for i in range(4)]

            for i in range(4):
                nc.sync.dma_start(xT_sb[i], xT[128 * i:128 * i + 128, :])
                nc.sync.dma_start(Wq_sb[i], Wq[128 * i:128 * i + 128, :])
                nc.sync.dma_start(Wk_sb[i], Wk[128 * i:128 * i + 128, :])
            nc.sync.dma_start(TR_sb, TR8[:, :])
            nc.sync.dma_start(id_sb, Id8[:, :])
            for i in range(4):
                nc.sync.dma_start(Wv_sb[i], Wv[128 * i:128 * i + 128, :])
            for i in range(4):
                nc.sync.dma_start(Wo_sb[i], Wo[128 * i:128 * i + 128, :])
            nc.sync.dma_start(bo_sb, bo_b[:, :])

            # ---- projections ----
            # qT/kT = W^T @ x^T  ([d, n] layout); q8 = unscaled q in fp8
            def proj_qk(mi):
                for c in range(2):
                    pqk = pso.tile([128, N], F32, name="pqk", tag="psum_o")
                    pq, pk = pqk[:, 0:512], pqk[:, 512:1024]
                    for ki in range(4):
                        f = dict(start=(ki == 0), stop=(ki == 3))
                        nc.tensor.matmul(
                            pq, Wq_sb[ki][:, 128 * mi:128 * mi + 128],
                            xT_sb[ki][:, 512 * c:512 * c + 512], **f)
                        nc.tensor.matmul(
                            pk, Wk_sb[ki][:, 128 * mi:128 * mi + 128],
                            xT_sb[ki][:, 512 * c:512 * c + 512], **f)
                    cs = slice(512 * c, 512 * c + 512)
                    # q pre-scaled by 64^-0.5 (dots only; pos handled via q8)
                    nc.scalar.mul(qT_sb[mi][:, cs], pq, SCALE)
                    nc.vector.tensor_copy(kT_sb[mi][:, cs], pk)
                    if (mi + c) % 2 == 0:
                        nc.vector.tensor_copy(q8_sb[mi][:, cs], pq)
                    else:
                        nc.scalar.copy(q8_sb[mi][:, cs], pq)

            # v = x @ Wv, stored [r, 8 x (64 v | 1.0)] for the Z ones-column
            def proj_rest():
                for mi in (2, 3):
                    proj_qk(mi)
                    yield
                for rt in range(8):
                    nc.gpsimd.memset(v_sb[rt][:, :], 1.0)
                    pv_t = pso.tile([128, N], F32, name="pv_t", tag="psum_o")
                    pv = pv_t[:, 0:512]
                    for ki in range(4):
                        nc.tensor.matmul(
                            pv, xT_sb[ki][:, 128 * rt:128 * rt + 128],
                            Wv_sb[ki], start=(ki == 0), stop=(ki == 3))
                    vdst = bass.AP(v_sb[rt].tensor, v_sb[rt].offset,
                                   [[8 * 65, 128], [65, 8], [1, 64]])
                    vsrc = bass.AP(pv_t.tensor, pv_t.offset,
                                   [[1024, 128], [64, 8], [1, 64]])
                    if rt % 2 == 0:
                        nc.scalar.copy(vdst, vsrc)
                    else:
                        nc.vector.tensor_copy(vdst, vsrc)
                    yield

            # ---- attention over head pairs ----
            state = {}

            def s_phase(hp, standalone=False):
                """s_ext windows for both heads -> fp8 staging in DRAM.
                Generator: yields after each ni so it can be interleaved
                into an a_phase t-loop (keeps the shared PSUM pool rotating
                instead of hard-serializing phase boundaries).
                standalone: no concurrent exp work, so split the big copies
                evenly between scalar and vector."""
                st = state[hp] = {}
                st["sd"] = sdram.tile([128, SW2], F8, name="sd", tag="sdram")
                sb = stage.tile([128, SW2], F8, name="sbAB", tag="s_big")
                for ni in range(NT):
                    qhA = q8_sb[hp][0:64, 128 * ni:128 * ni + 128]
                    qhB = q8_sb[hp][64:128, 128 * ni:128 * ni + 128]
                    W0 = 896 - 128 * ni
                    pse2 = ps.tile([128, N], F32, name="pse2", tag="psum")
                    for ci, cw in ((0, 512), (1, 512), (2, 128)):
                        sl = slice(W0 + 512 * ci, W0 + 512 * ci + cw)
                        nc.tensor.matmul(pse2[:, 0:cw], qhA, TR_sb[0:64, sl])
                        nc.tensor.matmul(pse2[:, 512:512 + cw], qhB,
                                         TR_sb[64:128, sl])
                        dst = bass.AP(sb.tensor,
                                      sb.offset + WIN * ni + 512 * ci,
                                      [[SW2, 128], [SW, 2], [1, cw]])
                        src = bass.AP(pse2.tensor, pse2.offset,
                                      [[1024, 128], [512, 2], [1, cw]])
                        if standalone:
                            to_scalar = (ci == 1) or (ci == 2 and ni % 2)
                        else:
                            to_scalar = (ci == 1 and ni % 2 == 0) or \
                                (ci == 2 and ni % 2 == 1)
                        if to_scalar:
                            nc.scalar.copy(dst, src)
                        else:
                            nc.vector.tensor_copy(dst, src)
                    if ni == 3:
                        nc.sync.dma_start(st["sd"][:, 0:4608], sb[:, 0:4608])
                        nc.sync.dma_start(st["sd"][:, SW:SW + 4608],
                                          sb[:, SW:SW + 4608])
                    yield
                nc.sync.dma_start(st["sd"][:, 4608:SW], sb[:, 4608:SW])
                nc.sync.dma_start(st["sd"][:, SW + 4608:SW2],
                                  sb[:, SW + 4608:SW2])

            def g_phase(hp):
                """skew gather: P[h][a + 128g, r] as [128, 8*1024] per head."""
                st = state[hp]
                st["pb"] = []
                sd = st["sd"]
                for h in range(2):
                    pb = pbp.tile([128, NT * N], F8, name="pb", tag="pgat")
                    src = bass.AP(sd.tensor, sd.offset + SW * h + 127,
                                  [[SW2 - 1, 128], [WIN, NT], [1, N]])
                    nc.gpsimd.dma_start(pb, src)
                    st["pb"].append(pb)

            def a_phase(hp, feeder=None):
                """transposed logits + exp + attn@v (+Z) + drain.
                feeder: optional s_phase generator pumped once per t."""
                st = state[hp]
                oX = [None, None]
                pending = []

                def attn_emit(t, hx, et):
                    h = 2 * hp + hx
                    if t == 0:
                        oX[hx] = pso.tile([65, N], F32, name=f"oX{hx}",
                                          tag="psum_o")
                    f = dict(start=(t == 0), stop=(t == NT - 1))
                    for c in range(2):
                        cs = slice(512 * c, 512 * c + 512)
                        nc.tensor.matmul(oX[hx][:, cs],
                                         v_sb[t][:, 65 * h:65 * h + 65],
                                         et[:, cs], **f)

                for t in range(NT):
                    if feeder is not None:
                        next(feeder, None)
                    for hx in range(2):
                        pd = ps.tile([128, N], F32, name="pd", tag="psum")
                        kb = kT_sb[hp][64 * hx:64 * hx + 64,
                                       128 * t:128 * t + 128]
                        qb = qT_sb[hp][64 * hx:64 * hx + 64, :]
                        pb = st["pb"][hx]
                        for c in range(2):
                            cs = slice(512 * c, 512 * c + 512)
                            nc.tensor.matmul(pd[:, cs], kb, qb[:, cs],
                                             start=True, stop=False)
                            for g in range(4 * c, 4 * c + 4):
                                nc.tensor.matmul(
                                    pd[:, 128 * g:128 * g + 128],
                                    pb[:, 1024 * g + 128 * t:
                                       1024 * g + 128 * t + 128],
                                    id_sb, start=False, stop=True)
                        et = etp.tile([128, N], BF, name="et", tag="et")
                        nc.scalar.activation(et[:, 0:512], pd[:, 0:512], EXP)
                        nc.scalar.activation(et[:, 512:1024],
                                             pd[:, 512:1024], EXP)
                        pending.append((t, hx, et))
                        if len(pending) > 4:
                            attn_emit(*pending.pop(0))
                if feeder is not None:
                    for _ in feeder:
                        pass
                for ent in pending:
                    attn_emit(*ent)

                for hx in range(2):
                    # custom-DVE ops can't read partition-shifted (base 64)
                    # PSUM rows; stage Z on partition 0 of SBUF first
                    zc = work.tile([1, N], F32, name="zc", tag="zc")
                    nc.scalar.copy(zc, oX[hx][64:65, :])
                    zr = work.tile([1, N], F32, name="zr", tag="zr")
                    nc.vector.reciprocal_approx_fast(zr, zc)
                    zb = work.tile([64, N], F32, name="zb", tag="zb")
                    nc.gpsimd.partition_broadcast(zb, zr)
                    nc.vector.tensor_mul(
                        aoT_sb[hp][64 * hx:64 * hx + 64, :],
                        oX[hx][0:64, :], zb)
                del state[hp]["pb"]

            # software-pipelined emission across head pairs: dense proj
            # first (PE ramps to max p-state), then S0/S1, then A phases
            # with S2/S3 fed into their t-loops at ni granularity.
            proj_qk(0)
            proj_qk(1)
            for _ in proj_rest():
                pass
            def run(gen):
                for _ in gen:
                    pass
            run(s_phase(0, standalone=True))
            run(s_phase(1, standalone=True))
            g_phase(0)
            g_phase(1)
            a_phase(0, feeder=s_phase(2))
            g_phase(2)
            a_phase(1, feeder=s_phase(3))
            g_phase(3)
            a_phase(2)
            a_phase(3)

            # ---- output projection ----
            for ni in range(NT):
                po_t = ps.tile([128, N], F32, name="po_t", tag="psum")
                po = po_t[:, 0:512]
                for ki in range(4):
                    nc.tensor.matmul(
                        po, aoT_sb[ki][:, 128 * ni:128 * ni + 128], Wo_sb[ki],
                        start=(ki == 0), stop=(ki == 3))
                o_sb = work.tile([128, DIM], F32, name="o_sb", tag="o_sb")
                nc.vector.tensor_add(o_sb, po, bo_sb)
                nc.sync.dma_start(out[128 * ni:128 * ni + 128, :], o_sb)

    nc.finalize()
    return nc


def _prep(x, Wq, Wkv, Wo, bo, pos_table):
    xT = np.ascontiguousarray(x.transpose(0, 2, 1)).astype(BF_NP)
    Wq_b = np.ascontiguousarray(Wq).astype(BF_NP)
    Wk_b = np.ascontiguousarray(Wkv[:, :INNER]).astype(BF_NP)
    Wv_b = np.ascontiguousarray(Wkv[:, INNER:]).astype(BF_NP)
    Wo_b = np.ascontiguousarray(Wo).astype(BF_NP)
    c = np.arange(2048)
    TR_half = pos_table[1024 - np.clip(c - 511, 0, 1024), :].T  # [64, 2048]
    # staged s_ext = (q) . (16 T); the device transpose-matmul multiplies by
    # I/128, so the pos logits come out as q.T/8 = SCALE * (q.T)
    TR8_b = np.ascontiguousarray(
        np.concatenate([TR_half, TR_half], axis=0) * 16.0).astype(F8_NP)
    Id8_b = (np.eye(128) / 128.0).astype(F8_NP)
    bo_b = np.ascontiguousarray(
        np.broadcast_to(bo.astype(np.float32), (128, DIM)))
    return xT, Wq_b, Wk_b, Wv_b, Wo_b, TR8_b, bo_b, Id8_b


def kernel(x, Wq, Wkv, Wo, bo, pos_table, _trace=False):
    global LAST_RESULTS
    if _trace:
        _install_ntff_hook()
    if "nc" not in _CACHE:
        _CACHE["nc"] = build()
    nc = _CACHE["nc"]
    xT, Wq_b, Wk_b, Wv_b, Wo_b, TR8_b, bo_b, Id8_b = _prep(
        np.asarray(x), np.asarray(Wq), np.asarray(Wkv), np.asarray(Wo),
        np.asarray(bo), np.asarray(pos_table))
    in_maps = [
        dict(xT=np.ascontiguousarray(xT[i]), Wq=Wq_b, Wk=Wk_b, Wv=Wv_b,
             Wo=Wo_b, bo_b=bo_b, TR8=TR8_b, Id8=Id8_b)
        for i in range(B)
    ]
    res = run_bass_kernel_spmd(nc, in_maps, core_ids=list(range(B)),
                               trace=_trace)
    LAST_RESULTS = res
    return np.stack([r["out"] for r in res.results], axis=0)


# revision 18
# speedup vs baseline: 1.0522x; 1.0522x over previous
"""Trainium2 Bass kernel for relative-position attention (dense_transformer).

Reference computation (per batch element b):
    q = x @ Wq; k, v = split(x @ Wkv); heads of 64
    dots = (q k^T) * 64^-0.5
    pos[n, r]  = (q[n] . pos_table[512 + clip(n - r, -512, 512)]) * 64^-0.5
    out = softmax(dots + pos) @ v; concat heads; @ Wo + bo

Sharding: pure data-parallel over the batch (B=8 -> 8 NeuronCores).

Design (v2, "transposed softmax"):
  * logits are computed TRANSPOSED: L^T[r, n] = k q^T + P^T, with the key
    index r on PSUM partitions.  exp(L^T) goes straight to SBUF as the
    attn@v operand -- the [1024,1024] E matrix never touches DRAM and no
    DMA transpose is needed (the old kernel spent ~34 MB of HBM on that).
  * relative-position term: s_ext = q @ TR (windowed, 1152 per 128-row
    tile) is staged to DRAM in FP8 (x16 scale) and skew-gathered back as
    P tiles [n, r] (1KB contiguous runs).  P^T is produced by fp8
    transpose-matmuls (lhsT = P tile, rhs = I/128) accumulating directly
    into the logits PSUM, so the skew costs no extra elementwise pass.
  * softmax denominator: v gets a ones-column (65-wide per head); the
    attn@v matmul then emits Z = sum_r E^T[r, n] as psum row 64.  The
    1/Z normalisation happens once on the [64, 1024] output drain
    (reciprocal + gpsimd partition_broadcast + one DVE multiply).
  * HBM traffic/core ~24 MB vs ~75 MB for the v1 kernel.
"""

import numpy as np
import ml_dtypes

import concourse.bass as bass
from concourse import bacc
import concourse.mybir as mybir
from concourse.tile import TileContext
from concourse.bass_utils import run_bass_kernel_spmd

B, N, DIM = 8, 1024, 512
HEADS, DH, INNER = 8, 64, 512
NT = N // 128            # 8 row tiles of 128
WIN = 1152               # s_ext window width per row tile
SW = NT * WIN            # 9216 staged cols per head
SW2 = 2 * SW             # 18432: A|B combined staging row
SCALE = DH ** -0.5
BF = mybir.dt.bfloat16
F32 = mybir.dt.float32
F8 = mybir.dt.float8e4
BF_NP = ml_dtypes.bfloat16
F8_NP = ml_dtypes.float8_e4m3

EXP = mybir.ActivationFunctionType.Exp

_CACHE = {}
LAST_RESULTS = None


def _install_ntff_hook():
    """The image's antenv package lacks axon_hooks; provide it so
    run_bass_kernel_spmd(trace=True) can capture NTFF profiles."""
    import sys
    import types
    if "antenv.axon_hooks" in sys.modules:
        return
    try:
        from trn_agent_boot.trn_boot import _ntff_profile_via_ctypes
        hook = _ntff_profile_via_ctypes("/opt/axon/libaxon_pjrt.so")
    except Exception:
        hook = None
    mod = types.ModuleType("antenv.axon_hooks")
    mod._hook = hook
    mod.set_axon_ntff_profile_hook = lambda h: setattr(mod, "_hook", h)
    mod.get_axon_ntff_profile_hook = lambda: mod._hook
    sys.modules["antenv.axon_hooks"] = mod


def build():
    nc = bacc.Bacc("TRN2")

    xT = nc.dram_tensor("xT", [DIM, N], BF, kind="ExternalInput")
    Wq = nc.dram_tensor("Wq", [DIM, INNER], BF, kind="ExternalInput")
    Wk = nc.dram_tensor("Wk", [DIM, INNER], BF, kind="ExternalInput")
    Wv = nc.dram_tensor("Wv", [DIM, INNER], BF, kind="ExternalInput")
    Wo = nc.dram_tensor("Wo", [INNER, DIM], BF, kind="ExternalInput")
    bo_b = nc.dram_tensor("bo_b", [128, DIM], F32, kind="ExternalInput")
    TR8 = nc.dram_tensor("TR8", [128, 2048], F8, kind="ExternalInput")
    Id8 = nc.dram_tensor("Id8", [128, 128], F8, kind="ExternalInput")
    out = nc.dram_tensor("out", [N, DIM], F32, kind="ExternalOutput")

    with TileContext(nc) as tc:
        with (
            tc.tile_pool(name="persist", bufs=1) as persist,
            tc.tile_pool(name="work", bufs=3) as work,
            tc.tile_pool(name="stage", bufs=2) as stage,
            tc.tile_pool(name="pbp", bufs=4) as pbp,
            tc.tile_pool(name="etp", bufs=8) as etp,
            tc.tile_pool(name="ps", bufs=2, space="PSUM") as ps,
            tc.tile_pool(name="pso", bufs=2, space="PSUM") as pso,
            tc.tile_pool(name="sdram", bufs=3, space="DRAM") as sdram,
        ):
            # ---- persistent SBUF tensors ----
            xT_sb = [persist.tile([128, N], BF, name=f"xT{i}") for i in range(4)]
            Wq_sb = [persist.tile([128, INNER], BF, name=f"Wq{i}") for i in range(4)]
            Wk_sb = [persist.tile([128, INNER], BF, name=f"Wk{i}") for i in range(4)]
            Wv_sb = [persist.tile([128, INNER], BF, name=f"Wv{i}") for i in range(4)]
            Wo_sb = [persist.tile([128, DIM], BF, name=f"Wo{i}") for i in range(4)]
            TR_sb = persist.tile([128, 2048], F8, name="TRt")
            bo_sb = persist.tile([128, DIM], F32, name="bot")
            id_sb = persist.tile([128, 128], F8, name="idt")
            qT_sb = [persist.tile([128, N], BF, name=f"qT{i}") for i in range(4)]
            q8_sb = [persist.tile([128, N], F8, name=f"q8{i}") for i in range(4)]
            kT_sb = [persist.tile([128, N], BF, name=f"kT{i}") for i in range(4)]
            v_sb = [persist.tile([128, 8 * 65], BF, name=f"v{i}") for i in range(8)]
            aoT_sb = [persist.tile([128, N], BF, name=f"aoT{i}") for i in range(4)]

            for i in range(4):
                nc.sync.dma_start(xT_sb[i], xT[128 * i:128 * i + 128, :])
                nc.sync.dma_start(Wq_sb[i], Wq[128 * i:128 * i + 128, :])
                nc.sync.dma_start(Wk_sb[i], Wk[128 * i:128 * i + 128, :])
            nc.sync.dma_start(TR_sb, TR8[:, :])
            nc.sync.dma_start(id_sb, Id8[:, :])
            for i in range(4):
                nc.sync.dma_start(Wv_sb[i], Wv[128 * i:128 * i + 128, :])
            for i in range(4):
                nc.sync.dma_start(Wo_sb[i], Wo[128 * i:128 * i + 128, :])
            nc.sync.dma_start(bo_sb, bo_b[:, :])

            # ---- projections ----
            # qT/kT = W^T @ x^T  ([d, n] layout); q8 = unscaled q in fp8
            def proj_qk(mi):
                for c in range(2):
                    pqk = pso.tile([128, N], F32, name="pqk", tag="psum_o")
                    pq, pk = pqk[:, 0:512], pqk[:, 512:1024]
                    for ki in range(4):
                        f = dict(start=(ki == 0), stop=(ki == 3))
                        nc.tensor.matmul(
                            pq, Wq_sb[ki][:, 128 * mi:128 * mi + 128],
                            xT_sb[ki][:, 512 * c:512 * c + 512], **f)
                        nc.tensor.matmul(
                            pk, Wk_sb[ki][:, 128 * mi:128 * mi + 128],
                            xT_sb[ki][:, 512 * c:512 * c + 512], **f)
                    cs = slice(512 * c, 512 * c + 512)
                    # q pre-scaled by 64^-0.5 (dots only; pos handled via q8)
                    nc.scalar.mul(qT_sb[mi][:, cs], pq, SCALE)
                    nc.vector.tensor_copy(kT_sb[mi][:, cs], pk)
                    if (mi + c) % 2 == 0:
                        nc.vector.tensor_copy(q8_sb[mi][:, cs], pq)
                    else:
                        nc.scalar.copy(q8_sb[mi][:, cs], pq)

            # v = x @ Wv, stored [r, 8 x (64 v | 1.0)] for the Z ones-column
            def proj_rest():
                for mi in (2, 3):
                    proj_qk(mi)
                    yield
                for rt in range(8):
                    nc.gpsimd.memset(v_sb[rt][:, :], 1.0)
                    pv_t = pso.tile([128, N], F32, name="pv_t", tag="psum_o")
                    pv = pv_t[:, 0:512]
                    for ki in range(4):
                        nc.tensor.matmul(
                            pv, xT_sb[ki][:, 128 * rt:128 * rt + 128],
                            Wv_sb[ki], start=(ki == 0), stop=(ki == 3))
                    vdst = bass.AP(v_sb[rt].tensor, v_sb[rt].offset,
                                   [[8 * 65, 128], [65, 8], [1, 64]])
                    vsrc = bass.AP(pv_t.tensor, pv_t.offset,
                                   [[1024, 128], [64, 8], [1, 64]])
                    if rt % 2 == 0:
                        nc.scalar.copy(vdst, vsrc)
                    else:
                        nc.vector.tensor_copy(vdst, vsrc)
                    yield

            # ---- attention over head pairs ----
            state = {}

            def s_phase(hp, standalone=False):
                """s_ext windows for both heads -> fp8 staging in DRAM.
                Generator: yields after each ni so it can be interleaved
                into an a_phase t-loop (keeps the shared PSUM pool rotating
                instead of hard-serializing phase boundaries).
                standalone: no concurrent exp work, so split the big copies
                evenly between scalar and vector."""
                st = state[hp] = {}
                st["sd"] = sdram.tile([128, SW2], F8, name="sd", tag="sdram")
                sb = stage.tile([128, SW2], F8, name="sbAB", tag="s_big")
                for ni in range(NT):
                    qhA = q8_sb[hp][0:64, 128 * ni:128 * ni + 128]
                    qhB = q8_sb[hp][64:128, 128 * ni:128 * ni + 128]
                    W0 = 896 - 128 * ni
                    pse2 = ps.tile([128, N], F32, name="pse2", tag="psum")
                    for ci, cw in ((0, 512), (1, 512), (2, 128)):
                        sl = slice(W0 + 512 * ci, W0 + 512 * ci + cw)
                        nc.tensor.matmul(pse2[:, 0:cw], qhA, TR_sb[0:64, sl])
                        nc.tensor.matmul(pse2[:, 512:512 + cw], qhB,
                                         TR_sb[64:128, sl])
                        dst = bass.AP(sb.tensor,
                                      sb.offset + WIN * ni + 512 * ci,
                                      [[SW2, 128], [SW, 2], [1, cw]])
                        src = bass.AP(pse2.tensor, pse2.offset,
                                      [[1024, 128], [512, 2], [1, cw]])
                        if standalone:
                            to_scalar = (ci == 1) or (ci == 2 and ni % 2)
                        else:
                            to_scalar = (ci == 1 and ni % 2 == 0) or \
                                (ci == 2 and ni % 2 == 1)
                        if to_scalar:
                            nc.scalar.copy(dst, src)
                        else:
                            nc.vector.tensor_copy(dst, src)
                    if ni == 3:
                        nc.sync.dma_start(st["sd"][:, 0:4608], sb[:, 0:4608])
                        nc.sync.dma_start(st["sd"][:, SW:SW + 4608],
                                          sb[:, SW:SW + 4608])
                    yield
                nc.sync.dma_start(st["sd"][:, 4608:SW], sb[:, 4608:SW])
                nc.sync.dma_start(st["sd"][:, SW + 4608:SW2],
                                  sb[:, SW + 4608:SW2])

            def g_phase(hp):
                """skew gather: P[h][a + 128g, r] as [128, 8*1024] per head."""
                st = state[hp]
                st["pb"] = []
                sd = st["sd"]
                for h in range(2):
                    pb = pbp.tile([128, NT * N], F8, name="pb", tag="pgat")
                    src = bass.AP(sd.tensor, sd.offset + SW * h + 127,
                                  [[SW2 - 1, 128], [WIN, NT], [1, N]])
                    nc.gpsimd.dma_start(pb, src)
                    st["pb"].append(pb)

            def a_phase(hp, feeder=None):
                """transposed logits + exp + attn@v (+Z) + drain.
                feeder: optional s_phase generator pumped once per t."""
                st = state[hp]
                oX = [None, None]
                pending = []

                def attn_emit(t, hx, et):
                    h = 2 * hp + hx
                    if t == 0:
                        oX[hx] = pso.tile([65, N], F32, name=f"oX{hx}",
                                          tag="psum_o")
                    f = dict(start=(t == 0), stop=(t == NT - 1))
                    for c in range(2):
                        cs = slice(512 * c, 512 * c + 512)
                        nc.tensor.matmul(oX[hx][:, cs],
                                         v_sb[t][:, 65 * h:65 * h + 65],
                                         et[:, cs], **f)

                for t in range(NT):
                    if feeder is not None:
                        next(feeder, None)
                    for hx in range(2):
                        pd = ps.tile([128, N], F32, name="pd", tag="psum")
                        kb = kT_sb[hp][64 * hx:64 * hx + 64,
                                       128 * t:128 * t + 128]
                        qb = qT_sb[hp][64 * hx:64 * hx + 64, :]
                        pb = st["pb"][hx]
                        for c in range(2):
                            cs = slice(512 * c, 512 * c + 512)
                            nc.tensor.matmul(pd[:, cs], kb, qb[:, cs],
                                             start=True, stop=False)
                            for g in range(4 * c, 4 * c + 4):
                                nc.tensor.matmul(
                                    pd[:, 128 * g:128 * g + 128],
                                    pb[:, 1024 * g + 128 * t:
                                       1024 * g + 128 * t + 128],
                                    id_sb, start=False, stop=True)
                        et = etp.tile([128, N], BF, name="et", tag="et")
                        nc.scalar.activation(et, pd, EXP)
                        pending.append((t, hx, et))
                        if len(pending) > 6:
                            attn_emit(*pending.pop(0))
                if feeder is not None:
                    for _ in feeder:
                        pass
                for ent in pending:
                    attn_emit(*ent)

                for hx in range(2):
                    # custom-DVE ops can't read partition-shifted (base 64)
                    # PSUM rows; stage Z on partition 0 of SBUF first
                    zc = work.tile([1, N], F32, name="zc", tag="zc")
                    nc.scalar.copy(zc, oX[hx][64:65, :])
                    zr = work.tile([1, N], F32, name="zr", tag="zr")
                    nc.vector.reciprocal_approx_fast(zr, zc)
                    zb = work.tile([64, N], F32, name="zb", tag="zb")
                    nc.gpsimd.partition_broadcast(zb, zr)
                    nc.vector.tensor_mul(
                        aoT_sb[hp][64 * hx:64 * hx + 64, :],
                        oX[hx][0:64, :], zb)
                del state[hp]["pb"]

            # software-pipelined emission across head pairs: dense proj
            # first (PE ramps to max p-state), then S0/S1, then A phases
            # with S2/S3 fed into their t-loops at ni granularity.
            proj_qk(0)
            proj_qk(1)
            for _ in proj_rest():
                pass
            def run(gen):
                for _ in gen:
                    pass
            run(s_phase(0, standalone=True))
            run(s_phase(1, standalone=True))
            g_phase(0)
            g_phase(1)
            a_phase(0, feeder=s_phase(2))
            g_phase(2)
            a_phase(1, feeder=s_phase(3))
            g_phase(3)
            a_phase(2)
            a_phase(3)

            # ---- output projection ----
            for ni in range(NT):
                po_t = ps.tile([128, N], F32, name="po_t", tag="psum")
                po = po_t[:, 0:512]
                for ki in range(4):
                    nc.tensor.matmul(
                        po, aoT_sb[ki][:, 128 * ni:128 * ni + 128], Wo_sb[ki],
                        start=(ki == 0), stop=(ki == 3))
                o_sb = work.tile([128, DIM], F32, name="o_sb", tag="o_sb")
                nc.vector.tensor_add(o_sb, po, bo_sb)
                nc.sync.dma_start(out[128 * ni:128 * ni + 128, :], o_sb)

    nc.finalize()
    return nc


def _prep(x, Wq, Wkv, Wo, bo, pos_table):
    xT = np.ascontiguousarray(x.transpose(0, 2, 1)).astype(BF_NP)
    Wq_b = np.ascontiguousarray(Wq).astype(BF_NP)
    Wk_b = np.ascontiguousarray(Wkv[:, :INNER]).astype(BF_NP)
    Wv_b = np.ascontiguousarray(Wkv[:, INNER:]).astype(BF_NP)
    Wo_b = np.ascontiguousarray(Wo).astype(BF_NP)
    c = np.arange(2048)
    TR_half = pos_table[1024 - np.clip(c - 511, 0, 1024), :].T  # [64, 2048]
    # staged s_ext = (q) . (16 T); the device transpose-matmul multiplies by
    # I/128, so the pos logits come out as q.T/8 = SCALE * (q.T)
    TR8_b = np.ascontiguousarray(
        np.concatenate([TR_half, TR_half], axis=0) * 16.0).astype(F8_NP)
    Id8_b = (np.eye(128) / 128.0).astype(F8_NP)
    bo_b = np.ascontiguousarray(
        np.broadcast_to(bo.astype(np.float32), (128, DIM)))
    return xT, Wq_b, Wk_b, Wv_b, Wo_b, TR8_b, bo_b, Id8_b


def kernel(x, Wq, Wkv, Wo, bo, pos_table, _trace=False):
    global LAST_RESULTS
    if _trace:
        _install_ntff_hook()
    if "nc" not in _CACHE:
        _CACHE["nc"] = build()
    nc = _CACHE["nc"]
    xT, Wq_b, Wk_b, Wv_b, Wo_b, TR8_b, bo_b, Id8_b = _prep(
        np.asarray(x), np.asarray(Wq), np.asarray(Wkv), np.asarray(Wo),
        np.asarray(bo), np.asarray(pos_table))
    in_maps = [
        dict(xT=np.ascontiguousarray(xT[i]), Wq=Wq_b, Wk=Wk_b, Wv=Wv_b,
             Wo=Wo_b, bo_b=bo_b, TR8=TR8_b, Id8=Id8_b)
        for i in range(B)
    ]
    res = run_bass_kernel_spmd(nc, in_maps, core_ids=list(range(B)),
                               trace=_trace)
    LAST_RESULTS = res
    return np.stack([r["out"] for r in res.results], axis=0)
